# revision 1
# baseline (speedup 1.0000x reference)
"""4-layer GAT + MLP head on Trainium2, 8-core SPMD (dst-sharded graph parallel).

Strategy:
  * Nodes sharded across 8 cores (6250/core); edges sharded by destination.
  * Per layer: each core computes h = x @ W_aug for its node shard (W_aug has
    extra columns producing alpha_src / alpha_dst "for free"), writes a
    node-major table [h | alpha_s] to HBM, then an AllGather replicates the
    full 50000-row table on every core.
  * Aggregation: edges (sorted by dst block, split by src table half for
    int16 gather indices) are processed 128 at a time: dma_gather pulls
    h[src] rows, alpha = leaky_relu(a_s[src]+a_d[dst]), e = exp(alpha),
    Y = e * h[src], and a one-hot selection matrix S (built by is_equal
    against an iota row) turns the segment-sum into PSUM-accumulated
    matmuls: out_block = S^T @ Y, denom = S^T @ e.
  * Softmax max-subtraction is skipped (alphas are O(1) here; exp is safe and
    the result is mathematically identical).
  * ELU is computed as relu(y) + min(exp(y),1) - 1 with the "-1" folded into
    the next layer's matmul via a column-sum correction row.
"""

import math
from contextlib import ExitStack

import numpy as np

P = 128
NCORES = 8
ADW = 64  # ad-table row width (f32 elems) -> 256B gather granularity


class Cfg:
    def __init__(self, N=50000, F_IN=10, HS=64, split=32768, neg=0.2, bf16=True):
        self.N, self.F_IN, self.HS, self.split, self.neg = N, F_IN, HS, split, neg
        self.bf16 = bf16
        assert N % NCORES == 0
        self.NPC = N // NCORES
        self.NBLK = math.ceil(self.NPC / P)
        self.ADPAD = self.NPC              # pad row in ad table (= -1e30)
        self.ADROWS = self.NPC + 8
        # layers: (f_in, f_out_total, heads)
        self.layers = [
            (F_IN, 4 * HS, 4),
            (4 * HS, 8 * HS, 4),
            (8 * HS, 8 * HS, 4),
            (8 * HS, HS, 1),
        ]
        # gather-table row widths (elems, 256B-aligned)
        al = 128 if bf16 else 64
        self.RS = [((fo + h + al - 1) // al) * al for (_, fo, h) in self.layers]
        self.ADWE = 128 if bf16 else 64   # ad-table row elems (256B)


CFG = Cfg()


# ------------------------------------------------------------------ host prep

def prep_edges(cfg, edge_index):
    """Shard edges by dst across cores, bucket by (dst block, src half),
    pad each bucket to a multiple of 128 slots uniform across cores."""
    ei = np.asarray(edge_index)
    n = cfg.N
    src = np.concatenate([ei[0], np.arange(n, dtype=np.int64)]).astype(np.int64)
    dst = np.concatenate([ei[1], np.arange(n, dtype=np.int64)]).astype(np.int64)
    core = dst // cfg.NPC
    dloc = dst - core * cfg.NPC
    blk = dloc // P
    dlb = dloc - blk * P
    half = (src >= cfg.split).astype(np.int64)

    key = (core * cfg.NBLK + blk) * 2 + half
    cnt = np.bincount(key, minlength=NCORES * cfg.NBLK * 2).reshape(
        NCORES, cfg.NBLK, 2)
    nch = -(-cnt.max(axis=0) // P)                  # [NBLK, 2] chunks (maxed)
    Cb = nch.sum(axis=1)                            # [NBLK]
    chunk_base = np.concatenate([[0], np.cumsum(Cb)]).astype(np.int64)
    total_slots = int(Cb.sum()) * P

    gb = np.zeros((cfg.NBLK, 2), np.int64)          # slot base per (blk, half)
    gb[:, 0] = chunk_base[:-1] * P
    gb[:, 1] = gb[:, 0] + nch[:, 0] * P

    order = np.argsort(key, kind="stable")
    ks = key[order]
    starts = np.r_[0, np.flatnonzero(np.diff(ks)) + 1]
    run_id = np.zeros(len(ks), np.int64)
    run_id[starts[1:]] = 1
    run_id = np.cumsum(run_id)
    rank = np.arange(len(ks)) - starts[run_id]
    slot = gb[blk[order], half[order]] + rank
    co = core[order]

    gidx = np.zeros((NCORES, total_slots), np.int64)
    dlocf = np.full((NCORES, total_slots), -1.0, np.float32)
    adf = np.full((NCORES, total_slots), cfg.ADPAD, np.int64)
    gidx[co, slot] = np.where(half[order] == 1, src[order] - cfg.split, src[order])
    dlocf[co, slot] = dlb[order].astype(np.float32)
    adf[co, slot] = dloc[order]

    def pack16(a):  # [NCORES, S] -> [NCORES, 128, S/16] int16, replicated x8
        s = a.shape[1]
        b = a.reshape(NCORES, s // 16, 16).transpose(0, 2, 1)
        return np.ascontiguousarray(np.tile(b, (1, 8, 1))).astype(np.int16)

    def pack128(a):  # [NCORES, S] -> [NCORES, 128, S/128]
        s = a.shape[1]
        return np.ascontiguousarray(
            a.reshape(NCORES, s // 128, P).transpose(0, 2, 1))

    return dict(
        nch=nch, chunk_base=chunk_base, gb=gb, total_slots=total_slots,
        gidx=pack16(gidx), adix=pack16(adf), dloc=pack128(dlocf))


def prep_weights(cfg, inputs):
    """Augment weights with alpha columns; compute corrections and biases."""
    out = {}
    names = [("W1", "as1", "ad1", "b1"), ("W2", "as2", "ad2", "b2"),
             ("W3", "as3", "ad3", "b3"), ("W4", "as4", "ad4", "b4")]
    for li, (wn, sn, dn, bn) in enumerate(names):
        fi, fo, h = cfg.layers[li]
        ch = fo // h
        W = np.asarray(inputs[wn], np.float32)
        a_s = np.asarray(inputs[sn], np.float32)
        a_d = np.asarray(inputs[dn], np.float32)
        As = np.zeros((fo, h), np.float32)
        Ad = np.zeros((fo, h), np.float32)
        for hh in range(h):
            As[hh * ch:(hh + 1) * ch, hh] = a_s[hh]
            Ad[hh * ch:(hh + 1) * ch, hh] = a_d[hh]
        waug = np.concatenate([W, W @ As, W @ Ad], axis=1)  # [fi, fo+2h]
        out[f"w{li}"] = waug
        if li > 0:  # input is elu(y)+1; subtract column sums
            out[f"cor{li}"] = np.tile(waug.sum(axis=0)[None, :], (P, 1)).astype(np.float32)
        b = np.asarray(inputs[bn], np.float32)
        out[f"bias{li}_nz"] = bool(np.any(b != 0))
        if out[f"bias{li}_nz"]:
            out[f"bias{li}"] = np.tile(b[None, :], (P, 1)).astype(np.float32)
    wm1 = np.asarray(inputs["Wm1"], np.float32)
    out["wm1"] = wm1
    out["wm2"] = np.asarray(inputs["Wm2"], np.float32)
    out["wm3"] = np.asarray(inputs["Wm3"], np.float32)
    bm1e = np.asarray(inputs["bm1"], np.float32) - wm1.sum(axis=0)
    out["bm1"] = np.tile(bm1e[None, :], (P, 1)).astype(np.float32)
    bm2 = np.asarray(inputs["bm2"], np.float32)
    out["bm2_nz"] = bool(np.any(bm2 != 0))
    if out["bm2_nz"]:
        out["bm2"] = np.tile(bm2[None, :], (P, 1)).astype(np.float32)
    out["bm3"] = np.full((P, 1), float(np.asarray(inputs["bm3"]).reshape(-1)[0]),
                         np.float32)
    return out


# --------------------------------------------------------------- bass program

def _install_queue_sem_patch():
    """Partition Tile's 8 DMASW sem lanes across the 4 SWDGE queues (2 each)
    so a sem is only ever incremented from one queue (runtime requirement)."""
    import concourse.tile_sem_assignment as tsa
    import concourse.mybir as mybir
    from concourse import bass_isa
    from concourse.tile_scheduler import DMAInst
    if getattr(tsa, "_q_aware", False):
        return
    orig = tsa.TileClockTick._assign_tick

    def _assign_tick_q(self, inst):
        if (isinstance(inst, DMAInst)
                and inst.engine == mybir.EngineType.Pool
                and not isinstance(inst, bass_isa.UserSyncedRemoteDMADescs)):
            q = getattr(inst, "queue_num", None) or 0
            cnt = getattr(self, "_q_cnt", None)
            if cnt is None:
                cnt = self._q_cnt = [0, 0, 0, 0]
            self.next_sw_dma_idx = 2 * q + (cnt[q] & 1)
            cnt[q] += 1
        return orig(self, inst)

    tsa.TileClockTick._assign_tick = _assign_tick_q
    tsa._q_aware = True


def build_program(cfg, ep, wmeta, ablate=(), stop_after=None, repeats=1):
    import concourse.bacc as bacc
    import concourse.mybir as mybir
    import concourse.tile as tile

    dt = mybir.dt
    f32 = dt.float32
    tdt = dt.bfloat16 if cfg.bf16 else f32
    ADWE = cfg.ADWE
    nch, chunk_base = ep["nch"], ep["chunk_base"]
    NBLK, NPC = cfg.NBLK, cfg.NPC
    NC16 = ep["gidx"].shape[2]
    NC128 = ep["dloc"].shape[2]
    HS = cfg.HS

    _install_queue_sem_patch()
    nc = bacc.Bacc("TRN2", target_bir_lowering=False, debug=False,
                   enable_asserts=False, num_devices=NCORES,
                   num_swdge_queues=4)
    T = {}

    def inp(name, shape, d=f32):
        T[name] = nc.dram_tensor(name, list(shape), d, kind="ExternalInput")
        return T[name]

    inp("x", [NPC, cfg.F_IN])
    inp("gidx", [P, NC16], dt.int16)
    inp("adix", [P, NC16], dt.int16)
    inp("dloc", [P, NC128])
    for li, (fi, fo, h) in enumerate(cfg.layers):
        inp(f"w{li}", [fi, fo + 2 * h])
        if li > 0:
            inp(f"cor{li}", [P, fo + 2 * h])
        if wmeta[f"bias{li}_nz"]:
            inp(f"bias{li}", [P, fo])
    inp("wm1", [HS, 4 * HS])
    inp("wm2", [4 * HS, 4 * HS])
    inp("wm3", [4 * HS, 1])
    inp("bm1", [P, 4 * HS])
    if wmeta["bm2_nz"]:
        inp("bm2", [P, 4 * HS])
    inp("bm3", [P, 1])
    inp("iota", [P, P])
    inp("ident", [P, P])
    inp("negrow", [1, ADWE], tdt)
    out_t = nc.dram_tensor("out", [NPC, 1], f32, kind="ExternalOutput")

    add, mult, sub = mybir.AluOpType.add, mybir.AluOpType.mult, mybir.AluOpType.subtract
    is_eq, vmax = mybir.AluOpType.is_equal, mybir.AluOpType.max
    EXP = mybir.ActivationFunctionType.Exp
    RELU = mybir.ActivationFunctionType.Relu
    SIGM = mybir.ActivationFunctionType.Sigmoid

    with tile.TileContext(nc) as tc, ExitStack() as ctx:
        const = ctx.enter_context(tc.tile_pool(name="const", bufs=1))
        dram = ctx.enter_context(tc.tile_pool(name="dram", bufs=1, space="DRAM"))
        ypool = ctx.enter_context(tc.tile_pool(name="y", bufs=2))
        apool = ctx.enter_context(tc.tile_pool(name="adg", bufs=2))
        spool = ctx.enter_context(tc.tile_pool(name="small", bufs=2))
        stpool = ctx.enter_context(tc.tile_pool(name="sel", bufs=3))
        bpool = ctx.enter_context(tc.tile_pool(name="blk", bufs=2))
        xpool = ctx.enter_context(tc.tile_pool(name="xT", bufs=3))
        psum = ctx.enter_context(tc.tile_pool(name="ps", bufs=1, space="PSUM"))

        # ---- constants into SBUF
        def load_const(name, shape, d=f32):
            t = const.tile(list(shape), d, tag=name)
            nc.sync.dma_start(t[:], T[name][tuple(slice(0, s) for s in shape)])
            return t

        iota_t = load_const("iota", [P, P])
        ident_t = load_const("ident", [P, P])
        gidx_t = load_const("gidx", [P, NC16], dt.int16)
        adix_t = load_const("adix", [P, NC16], dt.int16)
        dloc_t = load_const("dloc", [P, NC128])
        negrow_t = load_const("negrow", [1, ADWE], tdt)
        w_t, cor_t, bias_t = {}, {}, {}
        for li, (fi, fo, h) in enumerate(cfg.layers):
            kt = math.ceil(fi / P)
            pd = min(fi, P)
            wt = const.tile([pd, kt, fo + 2 * h], f32, tag=f"w{li}")
            nc.sync.dma_start(
                wt[:], T[f"w{li}"][:, :].rearrange("(k p) f -> p k f", p=pd))
            w_t[li] = wt
            if li > 0:
                cor_t[li] = load_const(f"cor{li}", [P, fo + 2 * h])
            if wmeta[f"bias{li}_nz"]:
                bias_t[li] = load_const(f"bias{li}", [P, fo])
        wm1_t = load_const("wm1", [HS, 4 * HS])
        wm2_t = const.tile([P, 2, 4 * HS], f32, tag="wm2")
        nc.sync.dma_start(wm2_t[:], T["wm2"][:, :].rearrange("(k p) f -> p k f", p=P))
        wm3_t = const.tile([P, 2, 1], f32, tag="wm3")
        nc.sync.dma_start(wm3_t[:], T["wm3"][:, :].rearrange("(k p) f -> p k f", p=P))
        bm1_t = load_const("bm1", [P, 4 * HS])
        bm2_t = load_const("bm2", [P, 4 * HS]) if wmeta["bm2_nz"] else None
        bm3_t = load_const("bm3", [P, 1])

        # ---- internal DRAM
        hloc = [dram.tile([NPC, cfg.RS[li]], tdt, name=f"hloc{li}", tag=f"hloc{li}")
                for li in range(4)]
        hfull = [None] * 4
        adloc = [dram.tile([cfg.ADROWS, ADWE], tdt, name=f"adloc{li}",
                           tag=f"adloc{li}")
                 for li in range(4)]

        def blocks():
            for b in range(NBLK):
                yield b, min(P, NPC - b * P)

        # ---------------- h table compute for layer li from x tile [pp, fi]
        def h_block(li, b, xp, pp):
            fi, fo, h = cfg.layers[li]
            kt = math.ceil(fi / P)
            psh = psum.tile([P, fo], f32, tag="psh", bufs=2)
            psa = psum.tile([P, 2 * h], f32, tag="psa")
            for k in range(kt):
                w = min(fi - k * P, P)
                ptr = psum.tile([P, P], f32, tag="ptr", bufs=2)
                nc.tensor.transpose(ptr[:w, :pp], xp[:pp, k * P:k * P + w],
                                    ident_t[:pp, :pp])
                xts = xpool.tile([P, P], f32, tag="xts")
                nc.vector.tensor_copy(xts[:w, :pp], ptr[:w, :pp])
                nc.tensor.matmul(psh[:pp, :], lhsT=xts[:w, :pp],
                                 rhs=w_t[li][:w, k, 0:fo],
                                 start=(k == 0), stop=(k == kt - 1))
                nc.tensor.matmul(psa[:pp, :], lhsT=xts[:w, :pp],
                                 rhs=w_t[li][:w, k, fo:fo + 2 * h],
                                 start=(k == 0), stop=(k == kt - 1))
            hrow = bpool.tile([P, cfg.RS[li]], tdt, tag="hrow")
            adrow = bpool.tile([P, ADWE], tdt, tag="adrow")
            nc.vector.memset(hrow[:pp, fo + h:cfg.RS[li]], 0.0)
            nc.vector.memset(adrow[:pp, h:ADWE], 0.0)
            if li > 0:
                nc.vector.tensor_tensor(hrow[:pp, 0:fo], psh[:pp, :],
                                        cor_t[li][:pp, 0:fo], sub)
                nc.vector.tensor_tensor(hrow[:pp, fo:fo + h], psa[:pp, 0:h],
                                        cor_t[li][:pp, fo:fo + h], sub)
                nc.vector.tensor_tensor(adrow[:pp, 0:h], psa[:pp, h:2 * h],
                                        cor_t[li][:pp, fo + h:fo + 2 * h], sub)
            else:
                nc.vector.tensor_copy(hrow[:pp, 0:fo], psh[:pp, :])
                nc.vector.tensor_copy(hrow[:pp, fo:fo + h], psa[:pp, 0:h])
                nc.vector.tensor_copy(adrow[:pp, 0:h], psa[:pp, h:2 * h])
            nc.sync.dma_start(hloc[li][b * P:b * P + pp, :], hrow[:pp, :])
            nc.sync.dma_start(adloc[li][b * P:b * P + pp, :], adrow[:pp, :])

        def allgather(li):
            nc.sync.dma_start(adloc[li][cfg.ADPAD:cfg.ADPAD + 1, :], negrow_t[:])
            nc.gpsimd.collective_compute(
                "AllGather", mybir.AluOpType.bypass,
                replica_groups=[list(range(NCORES))],
                ins=[hloc[li][:, :]], outs=[hfull[li][:, :]])

        ydum = {}
        edum = {}

        def get_dum(li):
            fi, fo, h = cfg.layers[li]
            Cmax = int(nch.sum(axis=1).max())
            if li not in ydum:
                yd = const.tile([P, Cmax, cfg.RS[li]], tdt, name=f"ydum{li}",
                                tag="ydum")
                nc.vector.memset(yd[:], 0.25)
                ydum[li] = yd
            return ydum[li], None

        # ---------------- aggregation for layer li, block b -> x' tile [pp, fo]
        def agg_block(li, b, pp):
            fi, fo, h = cfg.layers[li]
            ch = fo // h
            R = cfg.RS[li]
            nlo, nhi = int(nch[b, 0]), int(nch[b, 1])
            C = nlo + nhi
            cb = int(chunk_base[b])
            sbase = cb * P

            yt = ypool.tile([P, C, R], tdt, tag="yt")
            q0 = (b * 3) % 4
            if "nogather" in ablate:
                for cc in range(C):
                    nc.gpsimd.dma_gather(
                        yt[:, cc:cc + 1, :], hfull[li][0:cfg.split, :],
                        gidx_t[:, sbase // 16:(sbase + P) // 16],
                        P, P, R, single_packet=True)
            else:
                if nlo:
                    nc.gpsimd.dma_gather(
                        yt[:, 0:nlo, :], hfull[li][0:cfg.split, :],
                        gidx_t[:, sbase // 16:(sbase + nlo * P) // 16],
                        nlo * P, nlo * P, R, single_packet=(nlo * P <= 1024),
                        queue_num=q0)
                if nhi:
                    nc.gpsimd.dma_gather(
                        yt[:, nlo:C, :], hfull[li][cfg.split:cfg.N, :],
                        gidx_t[:, (sbase + nlo * P) // 16:(sbase + C * P) // 16],
                        nhi * P, nhi * P, R, single_packet=(nhi * P <= 1024),
                        queue_num=(q0 + 1) % 4)
            adg = apool.tile([P, C, ADWE], tdt, tag="adg")
            if "noadg" in ablate or "nogather" in ablate:
                nc.gpsimd.dma_gather(
                    adg[:, 0:1, :], adloc[li][:, :],
                    adix_t[:, sbase // 16:(sbase + P) // 16],
                    P, P, ADWE, single_packet=True, queue_num=(q0 + 2) % 4)
            else:
                nc.gpsimd.dma_gather(
                    adg[:], adloc[li][:, :],
                    adix_t[:, sbase // 16:(sbase + C * P) // 16],
                    C * P, C * P, ADWE, single_packet=(C * P <= 1024),
                    queue_num=(q0 + 2) % 4)

            if "nodep" in ablate:
                acc = spool.tile([P, 4], f32, tag="at0")
                nc.vector.tensor_tensor(acc[:], yt[:, 0, 0:4], adg[:, 0, 0:4], add)
                yt, adg = get_dum(li)[0][:, 0:C, :], adg
            if "gonly" in ablate:
                acc = spool.tile([P, 4], f32, tag="at")
                nc.vector.tensor_tensor(acc[:], yt[:, 0, 0:4], adg[:, 0, 0:4], add)
                xp = bpool.tile([P, fo], f32, tag="xp")
                nc.vector.memset(xp[:pp, :], 0.5)
                return xp
            at = spool.tile([P, C * h], f32, tag="at")
            if "noadg" in ablate or "nogather" in ablate:
                nc.vector.tensor_tensor(
                    at[:].rearrange("p (c h) -> p c h", h=h),
                    yt[:, :, fo:fo + h],
                    adg[:, 0:1, 0:h].to_broadcast([P, C, h]), add)
            else:
                nc.vector.tensor_tensor(
                    at[:].rearrange("p (c h) -> p c h", h=h),
                    yt[:, :, fo:fo + h], adg[:, :, 0:h], add)
            # leaky_relu: max(x, neg*x)
            at2 = spool.tile([P, C * h], f32, tag="at2")
            nc.vector.tensor_scalar_mul(at2[:], at[:], cfg.neg)
            nc.vector.tensor_tensor(at[:], at[:], at2[:], vmax)
            et = spool.tile([P, C * h], tdt, tag="et")
            nc.scalar.activation(et[:], at[:], EXP)
            # Y *= e  (in place, per head)
            nc.vector.tensor_tensor(
                yt[:, :, 0:fo].rearrange("p c (h ch) -> p c h ch", h=h),
                yt[:, :, 0:fo].rearrange("p c (h ch) -> p c h ch", h=h),
                et[:].rearrange("p (c h) -> p c h", h=h)
                     .unsqueeze(3).to_broadcast([P, C, h, ch]),
                mult)

            psy = psum.tile([P, fo], f32, tag="psy", bufs=2)
            pse = psum.tile([P, h], f32, tag="pse")
            G = 4
            for g0 in range(0, C, G):
                gg = min(G, C - g0)
                st = stpool.tile([P, G, P], tdt, tag="st")
                nc.vector.tensor_tensor(
                    st[:, 0:gg, :],
                    dloc_t[:, cb + g0:cb + g0 + gg].unsqueeze(2)
                          .to_broadcast([P, gg, P]),
                    iota_t[:].unsqueeze(1).to_broadcast([P, gg, P]),
                    is_eq)
                for c in range(g0, g0 + gg):
                    nc.tensor.matmul(psy[:, :], lhsT=st[:, c - g0, :],
                                     rhs=yt[:, c, 0:fo],
                                     start=(c == 0), stop=(c == C - 1))
                    nc.tensor.matmul(pse[:, :], lhsT=st[:, c - g0, :],
                                     rhs=et[:, c * h:(c + 1) * h],
                                     start=(c == 0), stop=(c == C - 1))
            # finish: x' = relu(y+b) + min(exp(y+b), 1)   (true x = x' - 1)
            den = spool.tile([P, h], f32, tag="den")
            nc.vector.tensor_scalar_add(den[:pp, :], pse[:pp, :], 1e-16)
            rec = spool.tile([P, h], f32, tag="rec")
            nc.vector.reciprocal(rec[:pp, :], den[:pp, :])
            ysb = bpool.tile([P, fo], f32, tag="ysb")
            nc.vector.tensor_tensor(
                ysb[:pp, :].rearrange("p (h ch) -> p h ch", h=h),
                psy[:pp, :].rearrange("p (h ch) -> p h ch", h=h),
                rec[:pp, :].unsqueeze(2).to_broadcast([pp, h, ch]), mult)
            if li in bias_t:
                nc.vector.tensor_tensor(ysb[:pp, :], ysb[:pp, :],
                                        bias_t[li][:pp, :], add)
            ex = bpool.tile([P, fo], f32, tag="ex")
            nc.scalar.activation(ex[:pp, :], ysb[:pp, :], EXP)
            nc.vector.tensor_scalar_min(ex[:pp, :], ex[:pp, :], 1.0)
            xp = bpool.tile([P, fo], f32, tag="xp")
            nc.vector.tensor_scalar_max(xp[:pp, :], ysb[:pp, :], 0.0)
            nc.vector.tensor_tensor(xp[:pp, :], xp[:pp, :], ex[:pp, :], add)
            return xp

        def mlp_block(b, xp, pp):
            # xp = x5' = x5+1 [pp, HS]; bm1 already corrected
            ptr = psum.tile([P, P], f32, tag="ptr", bufs=2)
            nc.tensor.transpose(ptr[:HS, :pp], xp[:pp, 0:HS], ident_t[:pp, :pp])
            xts = xpool.tile([P, P], f32, tag="xts")
            nc.vector.tensor_copy(xts[:HS, :pp], ptr[:HS, :pp])
            ps1 = psum.tile([P, 4 * HS], f32, tag="psh", bufs=2)
            nc.tensor.matmul(ps1[:pp, :], lhsT=xts[:HS, :pp], rhs=wm1_t[:, :],
                             start=True, stop=True)
            r1 = bpool.tile([P, 4 * HS], f32, tag="r1")
            nc.vector.tensor_tensor(r1[:pp, :], ps1[:pp, :], bm1_t[:pp, :], add)
            nc.scalar.activation(r1[:pp, :], r1[:pp, :], RELU)
            ps2 = psum.tile([P, 4 * HS], f32, tag="psy", bufs=2)
            for k in range(2):
                ptr2 = psum.tile([P, P], f32, tag="ptr", bufs=2)
                nc.tensor.transpose(ptr2[:, :pp], r1[:pp, k * P:(k + 1) * P],
                                    ident_t[:pp, :pp])
                xts2 = xpool.tile([P, P], f32, tag="xts")
                nc.vector.tensor_copy(xts2[:, :pp], ptr2[:, :pp])
                nc.tensor.matmul(ps2[:pp, :], lhsT=xts2[:, :pp],
                                 rhs=wm2_t[:, k, :], start=(k == 0), stop=(k == 1))
            r2 = bpool.tile([P, 4 * HS], f32, tag="r2")
            if bm2_t is not None:
                nc.vector.tensor_tensor(r2[:pp, :], ps2[:pp, :], bm2_t[:pp, :], add)
                nc.scalar.activation(r2[:pp, :], r2[:pp, :], RELU)
            else:
                nc.scalar.activation(r2[:pp, :], ps2[:pp, :], RELU)
            ps3 = psum.tile([P, 1], f32, tag="psa")
            for k in range(2):
                ptr3 = psum.tile([P, P], f32, tag="ptr", bufs=2)
                nc.tensor.transpose(ptr3[:, :pp], r2[:pp, k * P:(k + 1) * P],
                                    ident_t[:pp, :pp])
                xts3 = xpool.tile([P, P], f32, tag="xts")
                nc.vector.tensor_copy(xts3[:, :pp], ptr3[:, :pp])
                nc.tensor.matmul(ps3[:pp, :], lhsT=xts3[:, :pp],
                                 rhs=wm3_t[:, k, :], start=(k == 0), stop=(k == 1))
            osb = bpool.tile([P, 1], f32, tag="osb")
            nc.scalar.activation(osb[:pp, :], ps3[:pp, :], SIGM, bias=bm3_t[:pp, :])
            nc.sync.dma_start(out_t[b * P:b * P + pp, :], osb[:pp, :])

        # ------------------------------------------------ program body
        for _rep in range(repeats):
         for li in range(4):
            hfull[li] = dram.tile([cfg.N, cfg.RS[li]], tdt, addr_space="Shared",
                                  name=f"hfull{li}_{_rep}", tag=f"hfull{li}_{_rep}")
         for b, pp in blocks():
            xb = bpool.tile([P, cfg.F_IN], f32, tag="xb")
            nc.sync.dma_start(xb[:pp, :], T["x"][b * P:b * P + pp, :])
            h_block(0, b, xb, pp)
         allgather(0)
         if stop_after == 0:
            for b, pp in blocks():
                z = bpool.tile([P, 1], f32, tag="osb")
                nc.sync.dma_start(z[:pp, :], T["x"][b * P:b * P + pp, 0:1])
                nc.sync.dma_start(out_t[b * P:b * P + pp, :], z[:pp, :])
         else:
            for li in range(4):
                for b, pp in blocks():
                    xp = agg_block(li, b, pp)
                    if li == stop_after:
                        osb = bpool.tile([P, 1], f32, tag="osb")
                        nc.scalar.activation(osb[:pp, :], xp[:pp, 0:1],
                                             mybir.ActivationFunctionType.Copy)
                        nc.sync.dma_start(out_t[b * P:b * P + pp, :], osb[:pp, :])
                    elif li < 3:
                        h_block(li + 1, b, xp, pp)
                    else:
                        mlp_block(b, xp, pp)
                if li == stop_after:
                    break
                if li < 3:
                    allgather(li + 1)

    nc.compile()
    return nc


# ------------------------------------------------------------------ execution

def make_in_maps(cfg, ep, w, inputs):
    x = np.asarray(inputs["x"], np.float32)
    iota = np.tile(np.arange(P, dtype=np.float32)[None, :], (P, 1))
    ident = np.eye(P, dtype=np.float32)
    import ml_dtypes
    ndt = ml_dtypes.bfloat16 if cfg.bf16 else np.float32
    negrow = np.full((1, cfg.ADWE), -1e30, ndt)
    in_maps = []
    for c in range(NCORES):
        m = dict(
            x=np.ascontiguousarray(x[c * cfg.NPC:(c + 1) * cfg.NPC]),
            gidx=ep["gidx"][c], adix=ep["adix"][c], dloc=ep["dloc"][c],
            iota=iota, ident=ident, negrow=negrow,
            wm1=w["wm1"], wm2=w["wm2"], wm3=w["wm3"],
            bm1=w["bm1"], bm3=w["bm3"])
        if w["bm2_nz"]:
            m["bm2"] = w["bm2"]
        for li in range(4):
            m[f"w{li}"] = w[f"w{li}"]
            if li > 0:
                m[f"cor{li}"] = w[f"cor{li}"]
            if w[f"bias{li}_nz"]:
                m[f"bias{li}"] = w[f"bias{li}"]
        in_maps.append(m)
    return in_maps


_CACHE = {}


def _get_compiled(cfg, inputs):
    ep = prep_edges(cfg, inputs["edge_index"])
    w = prep_weights(cfg, inputs)
    key = (ep["gidx"].tobytes(), w["bm2_nz"],
           tuple(w[f"bias{li}_nz"] for li in range(4)))
    ck = hash(key)
    if ck not in _CACHE:
        _CACHE[ck] = build_program(cfg, ep, w)
    return _CACHE[ck], ep, w


def kernel(**inputs):
    from concourse import bass_utils
    cfg = CFG
    nc, ep, w = _get_compiled(cfg, inputs)
    in_maps = make_in_maps(cfg, ep, w, inputs)
    res = bass_utils.run_bass_kernel_spmd(nc, in_maps, core_ids=list(range(NCORES)))
    out = np.concatenate([res.results[c]["out"] for c in range(NCORES)], axis=0)
    return out.astype(np.float32)

